# revision 12
# baseline (speedup 1.0000x reference)
"""CustomMultiMarginLoss (p=1, margin=1.0, mean reduction) on 8 NeuronCores.

Math: loss = mean_b( sum_{c != t_b} max(0, 1 - (x[b,t_b] - x[b,c])) )
The excluded target column would contribute exactly relu(1) = 1, so
    loss = (1/B) * sum_b sum_c relu(x[b,c] + (1 - x[b,t_b])) - 1
which turns the whole problem into a streaming relu-with-per-row-bias plus a
row reduction: one fused instruction per tile (ACT `activation(Relu, bias,
accum_out)` / DVE `scalar_tensor_tensor(add, max, accum_out)`), written
in-place over the input tile so no extra SBUF is spent on the elementwise
result.

Sharding: data parallel over the batch dim. Core k owns rows
[k*1024, (k+1)*1024), processed as 8 blocks of 128 rows (rows on SBUF
partitions), streaming the 32000-wide class dim in W-wide chunks
(deep-buffered HWDGE DMAs, optionally split across both physical HWDGE
rings — SP `qSPDynamicHW` and ACT `qActDynamicHW`). Per-(block, chunk)
row-sums land in accumulator columns; the host sums the 8 per-core
[128, ncol] partials in float64 (the "all-reduce") and applies the /B and
-1 corrections.
"""

import numpy as np

B = 8192
C = 32000
NCORES = 8
ROWS_PER_CORE = B // NCORES  # 1024
P = 128
NBLK = ROWS_PER_CORE // P  # 8 blocks of 128 rows per core
W = 8000  # chunk width: 128 * 8000 * 4B = 4 MiB per DMA
BUFS = 6  # x-tile slots: 6 * 32 KiB = 192 KiB/partition
RINGS = ("sync", "scalar")  # alternate chunks across both physical HWDGE rings
FLAT = False  # contiguous-DRAM tiling (partition p <- p-th consecutive chunk)

_CACHE: dict = {}


def _build_program(
    repeat: int = 1,
    w: int = W,
    bufs: int = BUFS,
    rings: tuple = RINGS,
    inplace: bool = True,
    flat: bool = FLAT,
    loop: int = 1,
    split: bool = False,
):
    # repeat>1 duplicates the streaming body (re-reading the same input) —
    # used only for benchmarking to separate HW exec time from dispatch
    # overhead via the slope of time vs repeat. w/bufs/rings are
    # benchmarking knobs for chunk width, buffer depth, and DMA-ring split.
    import concourse.bacc as bacc
    import concourse.mybir as mybir
    from concourse.tile import TileContext

    nchunk = C // w
    ntile = NBLK * nchunk
    # split: ACT and DVE each reduce half of every tile -> 2 cols per tile
    ncol = ntile * (2 if split else 1)
    nbias = ((ntile if flat else NBLK)) + 1
    assert not (split and flat)

    f32 = mybir.dt.float32
    nc = bacc.Bacc(None, target_bir_lowering=False)
    inp = nc.dram_tensor("inp", [ROWS_PER_CORE, C], f32, kind="ExternalInput")
    # Last column is a host-supplied 0.0 (used as DVE max-operand), so no
    # device-side memset is needed.
    bias = nc.dram_tensor("bias", [P, nbias], f32, kind="ExternalInput")
    out = nc.dram_tensor("out", [P, ncol], f32, kind="ExternalOutput")

    if flat:
        # Tile nt = one contiguous P*w-element slab of the flat shard;
        # partition p takes the p-th consecutive w-chunk (w divides C, so
        # each chunk stays within one batch row -> per-partition bias).
        inp_r = inp.rearrange("(nt pr) (pc w) -> nt (pr pc) w", pr=P * w // C, w=w)
    else:
        inp_r = inp.rearrange("(nb p) c -> nb p c", p=P)  # [NBLK, 128, C]

    with TileContext(nc) as tc:
        with (
            tc.tile_pool(name="x", bufs=bufs) as xpool,
            tc.tile_pool(name="misc", bufs=1) as misc,
        ):
            bias_t = misc.tile([P, nbias], f32)
            nc.sync.dma_start(bias_t[:], bias[:, :])
            zeros = bias_t[:, nbias - 1 : nbias]
            acc = misc.tile([P, ncol], f32)  # even cols ACT, odd cols DVE
            if inplace:
                dummy_a = dummy_v = None
            else:
                dummy_a = misc.tile([P, w], f32)
                dummy_v = misc.tile([P, w], f32)

            def body():
                for j in range(NBLK * repeat):
                    j = j % NBLK
                    for i in range(nchunk):
                        col = j * nchunk + i
                        xt = xpool.tile([P, w], f32)
                        dma_eng = getattr(nc, rings[col % len(rings)])
                        if flat:
                            dma_eng.dma_start(xt[:], inp_r[col, :, :])
                            bj = bias_t[:, col : col + 1]
                        else:
                            dma_eng.dma_start(xt[:], inp_r[j, :, i * w : (i + 1) * w])
                            bj = bias_t[:, j : j + 1]
                        h = w // 2 if split else w
                        xa = xt[:, :h] if split else xt[:]
                        xv = xt[:, h:] if split else xt[:]
                        ca = 2 * col if split else col
                        cv = 2 * col + 1 if split else col
                        if split or col % 2 == 0:
                            nc.scalar.activation(
                                xa if inplace else dummy_a[:, :h],
                                xa,
                                mybir.ActivationFunctionType.Relu,
                                bias=bj,
                                scale=1.0,
                                accum_out=acc[:, ca : ca + 1],
                            )
                        if split or col % 2 == 1:
                            nc.vector.scalar_tensor_tensor(
                                out=xv if inplace else dummy_v[:, :h],
                                in0=xv,
                                scalar=bj,
                                in1=zeros.broadcast_to((P, h)),
                                op0=mybir.AluOpType.add,
                                op1=mybir.AluOpType.max,
                                accum_out=acc[:, cv : cv + 1],
                            )

            if loop > 1:
                # hardware loop (benchmarking only): re-executes the body —
                # accum_out columns are overwritten each iteration, so the
                # final out DMA reads the last iteration's (identical) values.
                with tc.For_i(0, loop):
                    body()
            else:
                body()

            nc.sync.dma_start(out[:], acc[:])

    nc.finalize()
    return nc


def _get_program():
    if "nc" not in _CACHE:
        _CACHE["nc"] = _build_program()
    return _CACHE["nc"]


def _make_in_maps(x: np.ndarray, t: np.ndarray, w: int = W, flat: bool = FLAT) -> list:
    # Per-row correct-class score and relu bias, computed during shard prep.
    correct = x[np.arange(B), t]  # [B] f32
    bias_full = (np.float32(1.0) - correct).astype(np.float32)

    ncol = ROWS_PER_CORE * C // (P * w)
    pc = C // w  # chunks per row == partitions per row in flat mode

    in_maps = []
    for k in range(NCORES):
        r0 = k * ROWS_PER_CORE
        shard = x[r0 : r0 + ROWS_PER_CORE]
        bf = bias_full[r0 : r0 + ROWS_PER_CORE]
        if flat:
            # bias_core[p, t] = bias of the row containing partition p's
            # chunk of tile t: row = t*(P//pc) + p//pc
            bias_core = np.zeros((P, ncol + 1), dtype=np.float32)
            rows = (np.arange(ncol)[None, :] * (P // pc)
                    + (np.arange(P) // pc)[:, None])  # [P, ncol]
            bias_core[:, :ncol] = bf[rows]
        else:
            bias_core = np.zeros((P, NBLK + 1), dtype=np.float32)
            bias_core[:, :NBLK] = bf.reshape(NBLK, P).T
        in_maps.append({"inp": shard, "bias": bias_core})
    return in_maps


def kernel(input: np.ndarray, target: np.ndarray, _results_out: list | None = None):
    from concourse.bass_utils import run_bass_kernel_spmd

    x = np.ascontiguousarray(np.asarray(input, dtype=np.float32))
    t = np.asarray(target).astype(np.int64)

    nc = _get_program()
    in_maps = _make_in_maps(x, t)

    res = run_bass_kernel_spmd(nc, in_maps, core_ids=list(range(NCORES)))
    if _results_out is not None:
        _results_out.append(res)

    total = np.float64(0.0)
    for k in range(NCORES):
        total += res.results[k]["out"].astype(np.float64).sum()

    loss = total / np.float64(B) - np.float64(1.0)
    return np.array(loss, dtype=np.float32)


# revision 14
# speedup vs baseline: 1.0136x; 1.0136x over previous
"""CustomMultiMarginLoss (p=1, margin=1.0, mean reduction) on 8 NeuronCores.

Math: loss = mean_b( sum_{c != t_b} max(0, 1 - (x[b,t_b] - x[b,c])) )
The excluded target column would contribute exactly relu(1) = 1, so
    loss = (1/B) * sum_b sum_c relu(x[b,c] + (1 - x[b,t_b])) - 1
which turns the whole problem into a streaming relu-with-per-row-bias plus a
row reduction: one fused instruction per tile (ACT `activation(Relu, bias,
accum_out)` / DVE `scalar_tensor_tensor(add, max, accum_out)`), written
in-place over the input tile so no extra SBUF is spent on the elementwise
result.

Sharding: data parallel over the batch dim. Core k owns rows
[k*1024, (k+1)*1024), processed as 8 blocks of 128 rows (rows on SBUF
partitions), streaming the 32000-wide class dim in W-wide chunks
(deep-buffered HWDGE DMAs, optionally split across both physical HWDGE
rings — SP `qSPDynamicHW` and ACT `qActDynamicHW`). Per-(block, chunk)
row-sums land in accumulator columns; the host sums the 8 per-core
[128, ncol] partials in float64 (the "all-reduce") and applies the /B and
-1 corrections.
"""

import numpy as np

B = 8192
C = 32000
NCORES = 8
ROWS_PER_CORE = B // NCORES  # 1024
P = 128
NBLK = ROWS_PER_CORE // P  # 8 blocks of 128 rows per core
W = 8000  # chunk width: 128 * 8000 * 4B = 4 MiB per DMA
BUFS = 4  # x-tile slots: 4 * 32 KiB = 128 KiB/partition (+2 x 32 KiB dummies)
RINGS = ("sync", "scalar")  # alternate chunks across both physical HWDGE rings
FLAT = False  # contiguous-DRAM tiling (partition p <- p-th consecutive chunk)
INPLACE = False  # separate elementwise-out dummies beat in-place on HW

_CACHE: dict = {}


def _build_program(
    repeat: int = 1,
    w: int = W,
    bufs: int = BUFS,
    rings: tuple = RINGS,
    inplace: bool = INPLACE,
    flat: bool = FLAT,
    loop: int = 1,
    split: bool = False,
):
    # repeat>1 duplicates the streaming body (re-reading the same input) —
    # used only for benchmarking to separate HW exec time from dispatch
    # overhead via the slope of time vs repeat. w/bufs/rings are
    # benchmarking knobs for chunk width, buffer depth, and DMA-ring split.
    import concourse.bacc as bacc
    import concourse.mybir as mybir
    from concourse.tile import TileContext

    nchunk = C // w
    ntile = NBLK * nchunk
    # split: ACT and DVE each reduce half of every tile -> 2 cols per tile
    ncol = ntile * (2 if split else 1)
    nbias = ((ntile if flat else NBLK)) + 1
    assert not (split and flat)

    f32 = mybir.dt.float32
    nc = bacc.Bacc(None, target_bir_lowering=False)
    inp = nc.dram_tensor("inp", [ROWS_PER_CORE, C], f32, kind="ExternalInput")
    # Last column is a host-supplied 0.0 (used as DVE max-operand), so no
    # device-side memset is needed.
    bias = nc.dram_tensor("bias", [P, nbias], f32, kind="ExternalInput")
    out = nc.dram_tensor("out", [P, ncol], f32, kind="ExternalOutput")

    if flat:
        # Tile nt = one contiguous P*w-element slab of the flat shard;
        # partition p takes the p-th consecutive w-chunk (w divides C, so
        # each chunk stays within one batch row -> per-partition bias).
        inp_r = inp.rearrange("(nt pr) (pc w) -> nt (pr pc) w", pr=P * w // C, w=w)
    else:
        inp_r = inp.rearrange("(nb p) c -> nb p c", p=P)  # [NBLK, 128, C]

    with TileContext(nc) as tc:
        with (
            tc.tile_pool(name="x", bufs=bufs) as xpool,
            tc.tile_pool(name="misc", bufs=1) as misc,
        ):
            bias_t = misc.tile([P, nbias], f32)
            nc.sync.dma_start(bias_t[:], bias[:, :])
            zeros = bias_t[:, nbias - 1 : nbias]
            acc = misc.tile([P, ncol], f32)  # even cols ACT, odd cols DVE
            if inplace:
                dummy_a = dummy_v = None
            else:
                dummy_a = misc.tile([P, w], f32)
                dummy_v = misc.tile([P, w], f32)

            def body():
                for j in range(NBLK * repeat):
                    j = j % NBLK
                    for i in range(nchunk):
                        col = j * nchunk + i
                        xt = xpool.tile([P, w], f32)
                        dma_eng = getattr(nc, rings[col % len(rings)])
                        if flat:
                            dma_eng.dma_start(xt[:], inp_r[col, :, :])
                            bj = bias_t[:, col : col + 1]
                        else:
                            dma_eng.dma_start(xt[:], inp_r[j, :, i * w : (i + 1) * w])
                            bj = bias_t[:, j : j + 1]
                        h = w // 2 if split else w
                        xa = xt[:, :h] if split else xt[:]
                        xv = xt[:, h:] if split else xt[:]
                        ca = 2 * col if split else col
                        cv = 2 * col + 1 if split else col
                        if split or col % 2 == 0:
                            nc.scalar.activation(
                                xa if inplace else dummy_a[:, :h],
                                xa,
                                mybir.ActivationFunctionType.Relu,
                                bias=bj,
                                scale=1.0,
                                accum_out=acc[:, ca : ca + 1],
                            )
                        if split or col % 2 == 1:
                            nc.vector.scalar_tensor_tensor(
                                out=xv if inplace else dummy_v[:, :h],
                                in0=xv,
                                scalar=bj,
                                in1=zeros.broadcast_to((P, h)),
                                op0=mybir.AluOpType.add,
                                op1=mybir.AluOpType.max,
                                accum_out=acc[:, cv : cv + 1],
                            )

            if loop > 1:
                # hardware loop (benchmarking only): re-executes the body —
                # accum_out columns are overwritten each iteration, so the
                # final out DMA reads the last iteration's (identical) values.
                with tc.For_i(0, loop):
                    body()
            else:
                body()

            nc.sync.dma_start(out[:], acc[:])

    nc.finalize()
    return nc


def _get_program():
    if "nc" not in _CACHE:
        _CACHE["nc"] = _build_program()
    return _CACHE["nc"]


def _make_in_maps(x: np.ndarray, t: np.ndarray, w: int = W, flat: bool = FLAT) -> list:
    # Per-row correct-class score and relu bias, computed during shard prep.
    correct = x[np.arange(B), t]  # [B] f32
    bias_full = (np.float32(1.0) - correct).astype(np.float32)

    ncol = ROWS_PER_CORE * C // (P * w)
    pc = C // w  # chunks per row == partitions per row in flat mode

    in_maps = []
    for k in range(NCORES):
        r0 = k * ROWS_PER_CORE
        shard = x[r0 : r0 + ROWS_PER_CORE]
        bf = bias_full[r0 : r0 + ROWS_PER_CORE]
        if flat:
            # bias_core[p, t] = bias of the row containing partition p's
            # chunk of tile t: row = t*(P//pc) + p//pc
            bias_core = np.zeros((P, ncol + 1), dtype=np.float32)
            rows = (np.arange(ncol)[None, :] * (P // pc)
                    + (np.arange(P) // pc)[:, None])  # [P, ncol]
            bias_core[:, :ncol] = bf[rows]
        else:
            bias_core = np.zeros((P, NBLK + 1), dtype=np.float32)
            bias_core[:, :NBLK] = bf.reshape(NBLK, P).T
        in_maps.append({"inp": shard, "bias": bias_core})
    return in_maps


def kernel(input: np.ndarray, target: np.ndarray, _results_out: list | None = None):
    from concourse.bass_utils import run_bass_kernel_spmd

    x = np.ascontiguousarray(np.asarray(input, dtype=np.float32))
    t = np.asarray(target).astype(np.int64)

    nc = _get_program()
    in_maps = _make_in_maps(x, t)

    res = run_bass_kernel_spmd(nc, in_maps, core_ids=list(range(NCORES)))
    if _results_out is not None:
        _results_out.append(res)

    total = np.float64(0.0)
    for k in range(NCORES):
        total += res.results[k]["out"].astype(np.float64).sum()

    loss = total / np.float64(B) - np.float64(1.0)
    return np.array(loss, dtype=np.float32)
